# revision 3
# baseline (speedup 1.0000x reference)
# Trainium2 Bass kernel v3 for nn_Attention_81028853007030
#
# Model: 1-unit LSTM over [B=64, L=2048, E=300] -> scores -> (buggy) mask ->
# softmax over L -> attn * x.
#
# v2 strategy (op-count-lean):
#   - fp16 x on the wire (in and out); host casts back to fp32. Validated
#     offline: rel err 2.7e-3 vs fp64 (tolerance 2e-2).
#   - Partition layout p = k*8 + s (k-major): 16 L-chunks x 8 seqs; each
#     partition owns 128 consecutive timesteps.
#   - Gates xg = x @ W'^T via PE: per tau, 3 fp16 transposes (overlapping
#     E-chunks 0:128/128:256/172:300 with zeroed W rows) + 3 matmuls,
#     PSUM->SBUF copies batched 2 taus/op.
#   - LSTM scan by fixed-point iteration (K=4): gates from h_{t-1} of the
#     previous sweep, c via ONE tensor_tensor_scan per sweep (state =
#     f*state + i*g along the free dim), h = o*tanh(c). Each partition
#     scans a 144-slot window (16 warmup + 128 real); warmup gates come
#     from the neighbor partition via one SBUF shift-DMA; zero gates fix
#     chunk 0 (zero state is a fixed point of zero gates).
#   - Softmax in the [128,128] layout: exp+accum fused, cross-chunk sum via
#     two tiny matmuls with 0/1 masks, all on-chip.
#   - attn*x multiply split DVE/Act/Pool, d-block out DMA, fp16.

import os as _os

import numpy as np

B, L, E = 64, 2048, 300
NCORES = 8
S = B // NCORES          # sequences per core
V = 128                  # partitions = 16 chunks x 8 seqs (k-major)
NCH = 16                 # L-chunks per sequence
WM = 16                  # warmup slots
WIN = WM + 128           # scan window per partition
K_ITERS = int(_os.environ.get("K_ITERS", "4"))
ECH = [0, 128, 172]      # E-chunk starts (each 128 wide; overlap handled in W)
NEG = -30.0              # masked score (exp(-30)*xmax ~ 1e-13, negligible)

# engine split for the 128 attn*x multiplies, per 8-tau d-block
MULT_DVE = int(_os.environ.get("MULT_DVE", "6"))
MULT_ACT = int(_os.environ.get("MULT_ACT", "2"))  # rest goes to gpsimd
# of the 32 xT copies (4 taus each), every COPY_ACT_EVERY-th goes to Act
COPY_ACT_EVERY = int(_os.environ.get("COPY_ACT_EVERY", "3"))
OUT_BLOCK = int(_os.environ.get("OUT_BLOCK", "16"))  # taus per out-DMA

_CACHE = {}


def _build_nc(loop_n=0):
    from contextlib import ExitStack

    import concourse.bacc as bacc
    import concourse.mybir as mybir
    from concourse import tile
    from concourse.masks import make_identity

    F32 = mybir.dt.float32
    F16 = mybir.dt.float16
    I32 = mybir.dt.int32
    Alu = mybir.AluOpType
    Act = mybir.ActivationFunctionType

    nc = bacc.Bacc("TRN2", target_bir_lowering=False, debug=False,
                   num_devices=NCORES)

    x_d = nc.dram_tensor("x", [S, L, E], F16, kind="ExternalInput")
    sl_d = nc.dram_tensor("sl", [S, 1], I32, kind="ExternalInput")
    wt_d = nc.dram_tensor("wt", [128, 12], F16, kind="ExternalInput")
    w4_d = nc.dram_tensor("w4", [1, 4], F32, kind="ExternalInput")
    b2r_d = nc.dram_tensor("b2r", [1, 32], F32, kind="ExternalInput")
    k16_d = nc.dram_tensor("k16", [128, 8], F32, kind="ExternalInput")
    k16t_d = nc.dram_tensor("k16t", [8, 128], F32, kind="ExternalInput")
    ksel_d = nc.dram_tensor("ksel", [8, 128], F32, kind="ExternalInput")
    m0_d = nc.dram_tensor("m0", [128, 1], F32, kind="ExternalInput")
    out_d = nc.dram_tensor("out", [S, L, E], F16, kind="ExternalOutput")

    # partition p = s*16+k covers timesteps k*128..k*128+127 of sequence s
    x_v = x_d.ap().rearrange("s (k t) e -> (s k) t e", t=128)
    out_v = out_d.ap().rearrange("s (k t) e -> (s k) t e", t=128)

    with tile.TileContext(nc) as tc, ExitStack() as ctx:
        cst = ctx.enter_context(tc.tile_pool(name="cst", bufs=1))
        big = ctx.enter_context(tc.tile_pool(name="big", bufs=1))
        xts = ctx.enter_context(tc.tile_pool(name="xts", bufs=3))
        ppxt = ctx.enter_context(tc.tile_pool(name="ppxt", bufs=2, space="PSUM"))
        ppxg = ctx.enter_context(tc.tile_pool(name="ppxg", bufs=2, space="PSUM"))
        ppm = ctx.enter_context(tc.tile_pool(name="ppm", bufs=1, space="PSUM"))

        # ---- constants (outside the loop) ----
        ident = cst.tile([128, 128], F16, tag="ident")
        make_identity(nc, ident[:])
        ones = cst.tile([1, 128], F32, tag="ones")
        nc.vector.memset(ones[:], 1.0)
        wt_sb = cst.tile([128, 3, 4], F16, tag="wt_sb")
        nc.sync.dma_start(wt_sb[:], wt_d.ap())
        k16_sb = cst.tile([128, 8], F32, tag="k16_sb")
        nc.sync.dma_start(k16_sb[:], k16_d.ap())
        k16t_sb = cst.tile([8, 128], F32, tag="k16t_sb")
        nc.sync.dma_start(k16t_sb[:], k16t_d.ap())
        sl_sb = cst.tile([S, 1], I32, tag="sl_sb")
        nc.sync.dma_start(sl_sb[:], sl_d.ap())
        w4_sb = cst.tile([1, 4], F32, tag="w4_sb")
        nc.sync.dma_start(w4_sb[:], w4_d.ap())
        b2r_sb = cst.tile([1, 32], F32, tag="b2r_sb")
        nc.sync.dma_start(b2r_sb[:], b2r_d.ap())

        # broadcasts to 128 partitions via ones^T @ row
        bc_ps = ppm.tile([128, 36], F32, tag="mps")
        nc.tensor.matmul(bc_ps[:, 0:32], lhsT=ones[:], rhs=b2r_sb[:],
                         start=True, stop=True)
        bconst8 = cst.tile([V, 4, 8], F32, tag="bconst8")
        nc.vector.tensor_copy(out=bconst8[:], in_=bc_ps[:, 0:32])
        w4_ps = ppm.tile([128, 36], F32, tag="mps")
        nc.tensor.matmul(w4_ps[:, 0:4], lhsT=ones[:], rhs=w4_sb[:],
                         start=True, stop=True)
        w4c = cst.tile([V, 4], F32, tag="w4c")
        nc.vector.tensor_copy(out=w4c[:], in_=w4_ps[:, 0:4])

        # kill0[s] = 1.0 if sl[s] > 0 else 0.0; killV[p] = kill0[s] at k==0
        slf = cst.tile([S, 1], F32, tag="slf")
        nc.vector.tensor_copy(out=slf[:], in_=sl_sb[:])
        kill0 = cst.tile([S, 1], F32, tag="kill0")
        nc.vector.tensor_scalar(kill0[:], slf[:], 0.0, None, Alu.is_gt)
        ksel_sb = cst.tile([8, 128], F32, tag="ksel_sb")
        nc.sync.dma_start(ksel_sb[:], ksel_d.ap())
        kv_ps = ppm.tile([128, 36], F32, tag="mps")
        nc.tensor.matmul(kv_ps[:, 0:1], lhsT=ksel_sb[:], rhs=kill0[:],
                         start=True, stop=True)
        killV = cst.tile([V, 1], F32, tag="killV")
        nc.vector.tensor_copy(out=killV[:], in_=kv_ps[:, 0:1])
        m0_sb = cst.tile([V, 1], F32, tag="m0_sb")
        nc.sync.dma_start(m0_sb[:], m0_d.ap())

        def emit_iter(it):
            x_sb = big.tile([V, 128, E], F16, tag=f"x_sb{it}")
            xga = big.tile([V, 4, WIN], F32, tag=f"xga{it}")
            hbuf = big.tile([V, 1 + WIN], F32, tag=f"hbuf{it}")
            p4 = big.tile([V, 4, WIN], F32, tag=f"p4{it}")
            g4 = big.tile([V, 4, WIN], F32, tag=f"g4{it}")
            ubuf = big.tile([V, WIN], F32, tag=f"ubuf{it}")
            cbuf = big.tile([V, WIN], F32, tag=f"cbuf{it}")
            thbuf = big.tile([V, WIN], F32, tag=f"thbuf{it}")
            attnn = big.tile([V, 128], F32, tag=f"attnn{it}")
            sums = big.tile([V, 1], F32, tag=f"sums{it}")
            z_sb = big.tile([S, 1], F32, tag=f"z_sb{it}")
            rinv = big.tile([S, 1], F32, tag=f"rinv{it}")

            # ---- input DMA + gates ----
            for d in range(16):
                nc.sync.dma_start(x_sb[:, d * 8:(d + 1) * 8, :],
                                  x_v[:, d * 8:(d + 1) * 8, :])

            def xg_quad(tau0):
                # transposes + matmuls for taus tau0..tau0+3
                xt_ps = ppxt.tile([128, 4, 3, 128], F16, tag="xt_ps")
                for i in range(4):
                    for c, e0 in enumerate(ECH):
                        nc.tensor.matmul(xt_ps[:, i, c, :],
                                         lhsT=x_sb[:, tau0 + i, e0:e0 + 128],
                                         rhs=ident[:], is_transpose=True,
                                         start=True, stop=True)
                xt_sb = xts.tile([128, 4, 3, 128], F16, tag="xt_sb")
                if (tau0 // 4) % COPY_ACT_EVERY == COPY_ACT_EVERY - 1:
                    nc.scalar.copy(out=xt_sb[:], in_=xt_ps[:])
                else:
                    nc.vector.tensor_copy(out=xt_sb[:], in_=xt_ps[:])
                return xt_sb

            for d in range(16):
                xg_ps = ppxg.tile([128, 8, 4], F32, tag="xg_ps")
                for quad in range(2):
                    tau0 = d * 8 + quad * 4
                    xt_sb = xg_quad(tau0)
                    for i in range(4):
                        for c in range(3):
                            nc.tensor.matmul(xg_ps[:, quad * 4 + i, :],
                                             lhsT=xt_sb[:, i, c, :],
                                             rhs=wt_sb[:, c, :],
                                             start=(c == 0), stop=(c == 2))
                # xga[:, j, WM+8d+t] = xg_ps[:, t, j] + b[j]
                nc.vector.scalar_tensor_tensor(
                    xga[:, :, WM + d * 8:WM + d * 8 + 8],
                    in0=xg_ps[:].rearrange("p t j -> p j t"),
                    scalar=1.0, in1=bconst8[:],
                    op0=Alu.mult, op1=Alu.add)

            # ---- warmup slots: shift from neighbor partition (chunk k-1
            # of seq s lives at p-1); k==0 partitions get zero gates, whose
            # fixed point is the zero state the sequence starts from.
            nc.sync.dma_start(xga[1:128, :, 0:WM],
                              xga[0:127, :, 128:WIN])
            nc.vector.tensor_scalar_mul(xga[:, :, 0:WM], xga[:, :, 0:WM],
                                        m0_sb[:])

            # ---- fixed-point iterations ----
            nc.vector.memset(hbuf[:, 0:1], 0.0)
            for k in range(K_ITERS):
                if k == 0:
                    gsrc = xga
                else:
                    for j in range(4):
                        nc.vector.scalar_tensor_tensor(
                            p4[:, j, :], in0=hbuf[:, 0:WIN],
                            scalar=w4c[:, j:j + 1], in1=xga[:, j, :],
                            op0=Alu.mult, op1=Alu.add)
                    gsrc = p4
                nc.scalar.activation(g4[:, 0:3, :], gsrc[:, 0:3, :],
                                     Act.Sigmoid)
                nc.scalar.activation(g4[:, 3, :], gsrc[:, 3, :], Act.Tanh)
                nc.vector.tensor_tensor(out=ubuf[:], in0=g4[:, 0, :],
                                        in1=g4[:, 3, :], op=Alu.mult)
                nc.vector.tensor_tensor_scan(
                    out=cbuf[:], data0=g4[:, 1, :], data1=ubuf[:],
                    initial=0.0, op0=Alu.mult, op1=Alu.add)
                nc.scalar.activation(thbuf[:], cbuf[:], Act.Tanh)
                nc.vector.tensor_tensor(out=hbuf[:, 1:1 + WIN],
                                        in0=g4[:, 2, :], in1=thbuf[:],
                                        op=Alu.mult)

            # ---- softmax over L per sequence ----
            # mask t=0: killV is kill0[s] on k==0 partitions, 0 elsewhere
            nc.vector.scalar_tensor_tensor(
                hbuf[:, 1 + WM:2 + WM], in0=killV[:], scalar=NEG,
                in1=hbuf[:, 1 + WM:2 + WM], op0=Alu.mult, op1=Alu.add)
            nc.scalar.activation(attnn[:], hbuf[:, 1 + WM:1 + WIN], Act.Exp,
                                 accum_out=sums[:])
            zps = ppm.tile([8, 4], F32, tag="mps")
            nc.tensor.matmul(zps[:, 0:1], lhsT=k16_sb[:], rhs=sums[:],
                             start=True, stop=True)
            nc.vector.tensor_copy(out=z_sb[:], in_=zps[:, 0:1])
            nc.vector.reciprocal(rinv[:], z_sb[:])
            rv_ps = ppm.tile([128, 4], F32, tag="mps")
            nc.tensor.matmul(rv_ps[:, 0:1], lhsT=k16t_sb[:], rhs=rinv[:],
                             start=True, stop=True)
            nc.vector.tensor_scalar_mul(attnn[:], attnn[:], rv_ps[:, 0:1])

            # ---- out = attn * x; out DMA on gpsimd (own DMA channel) ----
            for b in range(128 // OUT_BLOCK):
                for i in range(OUT_BLOCK):
                    tau = b * OUT_BLOCK + i
                    sc = attnn[:, tau:tau + 1]
                    if i % 8 < MULT_DVE:
                        nc.vector.tensor_scalar_mul(x_sb[:, tau, :],
                                                    x_sb[:, tau, :], sc)
                    elif i % 8 < MULT_DVE + MULT_ACT:
                        nc.scalar.activation(x_sb[:, tau, :], x_sb[:, tau, :],
                                             Act.Copy, scale=sc)
                    else:
                        nc.gpsimd.tensor_scalar_mul(x_sb[:, tau, :],
                                                    x_sb[:, tau, :], sc)
                nc.gpsimd.dma_start(
                    out_v[:, b * OUT_BLOCK:(b + 1) * OUT_BLOCK, :],
                    x_sb[:, b * OUT_BLOCK:(b + 1) * OUT_BLOCK, :])

        if loop_n:
            assert loop_n % 2 == 0
            with tc.For_i(0, loop_n // 2, 1):
                emit_iter(0)
                emit_iter(1)
        else:
            emit_iter(0)

    nc.compile()
    return nc


def _get_nc(loop_n=0):
    key = ("nc", loop_n, K_ITERS, MULT_DVE, MULT_ACT, COPY_ACT_EVERY, OUT_BLOCK)
    if key not in _CACHE:
        _CACHE[key] = _build_nc(loop_n)
    return _CACHE[key]


# gate order: pytorch (i,f,g,o) -> device (i,f,o,g)
_PERM = [0, 1, 3, 2]


def make_in_maps(x, source_lengths, W_ih, W_hh, b_ih, b_hh):
    x16 = np.asarray(x, dtype=np.float16)
    sl = np.asarray(source_lengths).astype(np.int32).reshape(B, 1)
    wp = np.asarray(W_ih, dtype=np.float32)[_PERM]          # [4, E]
    # wt[r, c, j] = W'[j, e0_c + r], rows overlapping a previous chunk zeroed
    wt = np.zeros((128, 3, 4), np.float32)
    for c, e0 in enumerate(ECH):
        wt[:, c, :] = wp[:, e0:e0 + 128].T
    wt[0:84, 2, :] = 0.0                                    # e 172..255 dup
    wt = wt.reshape(128, 12).astype(np.float16)
    w4 = np.asarray(W_hh, dtype=np.float32).reshape(4)[_PERM].reshape(1, 4)
    b2 = (np.asarray(b_ih, dtype=np.float32)
          + np.asarray(b_hh, dtype=np.float32))[_PERM]
    b2r = np.repeat(b2, 8).reshape(1, 32).astype(np.float32)  # [4,8] j-major
    k16 = np.zeros((128, 8), np.float32)
    k16[np.arange(128), np.arange(128) // 16] = 1.0
    k16t = np.ascontiguousarray(k16.T)
    ksel = np.zeros((8, 128), np.float32)
    ksel[np.arange(8), np.arange(8) * 16] = 1.0
    m0 = (np.arange(128) % 16 != 0).astype(np.float32).reshape(128, 1)
    in_maps = []
    for c in range(NCORES):
        in_maps.append({
            "x": np.ascontiguousarray(x16[c * S:(c + 1) * S]),
            "sl": np.ascontiguousarray(sl[c * S:(c + 1) * S]),
            "wt": wt,
            "w4": w4,
            "b2r": b2r,
            "k16": k16,
            "k16t": k16t,
            "ksel": ksel,
            "m0": m0,
        })
    return in_maps


def kernel(x, source_lengths, W_ih, W_hh, b_ih, b_hh):
    from concourse.bass_utils import run_bass_kernel_spmd

    nc = _get_nc()
    in_maps = make_in_maps(x, source_lengths, W_ih, W_hh, b_ih, b_hh)
    res = run_bass_kernel_spmd(nc, in_maps, core_ids=list(range(NCORES)))
    out = np.concatenate([res.results[c]["out"] for c in range(NCORES)],
                         axis=0)
    return out.astype(np.float32)
